# revision 28
# baseline (speedup 1.0000x reference)
"""Trainium2 Bass kernel for nn_Attention_52536039965434.

Reference computation (B=2, SQ=SK=2048, H=1024, NH=16, HD=64):
    qkv = x @ c_attn_w + b ; per-head attention with multiplicative mask
    (post-score, pre-softmax); attn @ c_proj_w + b; gelu(cat(x, attn) @ mlp_w + b)

Sharding (8 cores): core c -> (b = c//4, g = c%4). Data parallel over batch,
tensor parallel over 4 head-groups (4 heads = 256 dims each) for QKV +
attention. The tail is SEQUENCE parallel: each core computes a full-width
c_proj partial from its own heads (z_partial = attn_g @ c_proj_w[g rows, :]),
one ReduceScatter(add) over the 4-core group turns that into the exact
z^T for the core's 512-query slice, and the MLP runs locally on that slice
producing out^T [H, 512].

Masked-key elimination (exact): the reference mask is MULTIPLICATIVE on
scores pre-softmax, so masked keys have score 0 -> exp 0 = 1 -> they
contribute exp=1 to the denominator and 1*V_k to the numerator, a rank-1
term independent of the query. Attention is permutation-invariant over
keys, so the host reorders keys to put the ~50% unmasked ones first
(zero-padded to a multiple of 256 columns; a zero x column gives K=0,
V=0 -> contributes exp(0)*1 to the denominator only, which the host
subtracts from the correction term). The device runs scores/exp/PV over
only ceil(n_unmasked/256)*2 key tiles and adds the masked contribution
via one [1,65]-per-head matmul against a host-computed correction row:
    corr_head = [sum_masked V_k (64 dims), count_masked - n_padding]
This halves K/V compute, score matmuls, exp work, and PV matmuls with
NO approximation (same summation, reordered).

Other identities:
  - softmax without max-subtraction (scores are O(15), exp safe in f32)
  - denominator = 65th PV output column (V augmented with a ones column)
  - Q/K/V biases via an augmented contraction row; c_proj bias folded into
    the mlp bias on the host: mlp_b_eff = mlp_b + c_proj_b @ mlp_w[H:2H, :]

Schedule: stage 1 computes K^T + V for the compacted keys. Stage 2 is a
per-q-block pipeline: Q^T for the block, score chunks feeding exp (ACT)
feeding PV (software-pipelined one chunk behind), with c_proj partial
matmuls for the previous q-block as PE filler. The tail interleaves the
last q-block's c_proj with mlp1 (x-half of the MLP), whose PSUM
accumulators recycle the attention's PSUM slots; mlp2 accumulates into the
same banks after the ReduceScatter and drains through gelu.

DMA queues: x streaming / zin / zch / out on the SP HWDGE queue; weight
loads on the gpsimd SWDGE queue so they never head-of-line block the
streams.
"""

import os

import numpy as np

import concourse.bacc as bacc
import concourse.mybir as mybir
import concourse.tile as tile
from concourse import bass_utils

# ---- problem dims (hardcoded per contest contract) ----
B = 2
S = 2048          # SQ == SK
H = 1024
NH = 16
HD = 64
NCORES = 8
TP = 4            # cores per batch (head groups)
HPC = NH // TP    # heads per core = 4
DH = HPC * HD     # per-core head width = 256
QB = 512          # q-block (matmul moving free dim)
SQ4 = S // TP     # per-core output q-slice = 512
P = 128

F32 = mybir.dt.float32
F32R = mybir.dt.float32r
AF = mybir.ActivationFunctionType
ALU = mybir.AluOpType


def _build_nc(T, reps=1, cc_mode=None):
    """Build + compile the SPMD Bass program. T = number of 128-wide key
    tiles actually attended (even; host pads the compacted keys to T*128)."""
    if cc_mode is None:
        cc_mode = os.environ.get("KERNEL_CC", "cc")  # cc | dma (timing expt)

    nq = S // QB          # q blocks = 4
    nf = H // P           # feature tiles of H = 8
    SKP = T * P           # padded compacted key count

    nc = bacc.Bacc(
        "TRN2", target_bir_lowering=False, debug=False, num_devices=NCORES
    )

    # ---- kernel I/O (per-core contents supplied via in_maps) ----
    xatt = nc.dram_tensor("xatt", [H + 1, S], F32R, kind="ExternalInput").ap()
    xatd = nc.dram_tensor("xatd", [H + 1, SKP], F32R, kind="ExternalInput").ap()
    wq_d = nc.dram_tensor("wq", [H + 1, DH], F32R, kind="ExternalInput").ap()
    wk_d = nc.dram_tensor("wk", [H + 1, DH], F32R, kind="ExternalInput").ap()
    wv_d = nc.dram_tensor("wv", [H + 1, DH], F32R, kind="ExternalInput").ap()
    corr_d = nc.dram_tensor("corr", [1, 4 * 65], F32R, kind="ExternalInput").ap()
    cpw_d = nc.dram_tensor("cprojw", [DH, H], F32R, kind="ExternalInput").ap()
    mw1_d = nc.dram_tensor("mlpw1", [H + 1, H], F32R, kind="ExternalInput").ap()
    mw2_d = nc.dram_tensor("mlpw2", [H, H], F32R, kind="ExternalInput").ap()
    # this core's 512-query slice of attender_seq^T (incl. ones row), for mlp1
    xsl_d = nc.dram_tensor("xslice", [H + 1, SQ4], F32R, kind="ExternalInput").ap()
    outT = nc.dram_tensor("outT", [H, SQ4], F32, kind="ExternalOutput").ap()

    rg = [[0, 1, 2, 3], [4, 5, 6, 7]]

    with tile.TileContext(nc) as tc:
      for rep in range(reps):
        with (
            tc.tile_pool(name=f"dram{rep}", bufs=1, space="DRAM") as dram,
            tc.tile_pool(name=f"xstream{rep}", bufs=4) as xstream,
            tc.tile_pool(name=f"augstream{rep}", bufs=2) as augstream,
            tc.tile_pool(name=f"persist{rep}", bufs=1) as persist,
        ):
            # internal DRAM for the collective
            zin = dram.tile([TP * H, SQ4], F32, tag="zin", name="zin")
            zred = dram.tile([H, SQ4], F32, tag="zred", name="zred")

            # persistent attention state (lives through stage 2)
            wq_sb = persist.tile([P, nf * DH], F32R, tag="wq")
            wqb_sb = persist.tile([1, DH], F32R, tag="wqb")
            KT_sb = persist.tile([P, 2 * SKP], F32R, tag="kt")
            V_sb = persist.tile([P, T * 260], F32R, tag="v")  # kt: 4 h x 65
            corr_sb = persist.tile([1, 4 * 65], F32R, tag="corr")
            ones_sb = persist.tile([1, QB], F32R, tag="ones")
            nc.vector.memset(ones_sb[:].bitcast(F32), 1.0)
            nc.gpsimd.dma_start(out=corr_sb[:], in_=corr_d[:])

            # ---------- stage 1: K^T and V for compacted keys ----------
            with (
                tc.tile_pool(name=f"wkv{rep}", bufs=1) as wkv,
                tc.tile_pool(name=f"ps1{rep}", bufs=2, space="PSUM") as ps1,
            ):
                wk_sb = wkv.tile([P, nf * DH], F32R, tag="wk")
                wv_sb = wkv.tile([P, nf * DH], F32R, tag="wv")
                wkb_sb = wkv.tile([1, DH], F32R, tag="wkb")
                wvb_sb = wkv.tile([1, DH], F32R, tag="wvb")
                # wk first on the SP queue: it gates the very first matmul
                nc.sync.dma_start(
                    out=wk_sb[:].rearrange("p (t d) -> p t d", d=DH),
                    in_=wk_d[:H].rearrange("(t p) d -> p t d", p=P),
                )
                nc.sync.dma_start(out=wkb_sb[:], in_=wk_d[H : H + 1])
                for w_d, w_sb, wb_sb in (
                    (wv_d, wv_sb, wvb_sb),
                    (wq_d, wq_sb, wqb_sb),
                ):
                    nc.gpsimd.dma_start(
                        out=w_sb[:].rearrange("p (t d) -> p t d", d=DH),
                        in_=w_d[:H].rearrange("(t p) d -> p t d", p=P),
                    )
                    nc.gpsimd.dma_start(out=wb_sb[:], in_=w_d[H : H + 1])

                # ones columns of the augmented V (denominator trick)
                for kt in range(T):
                    nc.vector.memset(
                        V_sb[:, kt * 260 : (kt + 1) * 260]
                        .rearrange("p (h c) -> p h c", c=65)[:, :, 64:65]
                        .opt()
                        .bitcast(F32),
                        1.0,
                    )

                for off in range(0, SKP, QB):
                    w = min(QB, SKP - off)
                    kb = off // QB
                    x_ch = [
                        xstream.tile([P, (nf // 2) * QB], F32R, tag="xch",
                                     name=f"xd{kb}_{i}")
                        for i in range(2)
                    ]
                    x_aug = augstream.tile([1, QB], F32R, tag="xaug")
                    for i in range(2):
                        nc.sync.dma_start(
                            out=x_ch[i][:, : 4 * w].rearrange(
                                "p (t q) -> p t q", q=w),
                            in_=xatd[i * (H // 2) : (i + 1) * (H // 2)]
                            .rearrange("(t p) q -> p t q", p=P)
                            [:, :, off : off + w],
                        )
                    nc.sync.dma_start(
                        out=x_aug[:, :w], in_=xatd[H : H + 1, off : off + w]
                    )
                    for p in range(2):
                        ps = ps1.tile([P, QB], F32, tag="acc512")
                        for t in range(nf):
                            nc.tensor.matmul(
                                ps[:, :w],
                                lhsT=w_slice(wk_sb, t, p),
                                rhs=x_ch[t // 4][:, (t % 4) * w : (t % 4 + 1) * w],
                                start=(t == 0),
                                stop=False,
                            )
                        nc.tensor.matmul(
                            ps[:, :w],
                            lhsT=wkb_sb[0:1, p * P : (p + 1) * P],
                            rhs=x_aug[:, :w],
                            start=False,
                            stop=True,
                        )
                        nc.vector.tensor_copy(
                            KT_sb[:, p * SKP + off : p * SKP + off + w],
                            ps[:, :w],
                        )
                    for sub in range(w // P):
                        kt = off // P + sub
                        psv = ps1.tile([P, DH], F32, tag="accv")
                        for t in range(nf):
                            nc.tensor.matmul(
                                psv[:],
                                lhsT=x_ch[t // 4][
                                    :,
                                    (t % 4) * w + sub * P : (t % 4) * w
                                    + (sub + 1) * P,
                                ],
                                rhs=wv_sb[:, t * DH : (t + 1) * DH],
                                start=(t == 0),
                                stop=False,
                            )
                        nc.tensor.matmul(
                            psv[:],
                            lhsT=x_aug[0:1, sub * P : (sub + 1) * P],
                            rhs=wvb_sb[:],
                            start=False,
                            stop=True,
                        )
                        nc.vector.tensor_copy(
                            V_sb[:, kt * 260 : (kt + 1) * 260]
                            .rearrange("p (h c) -> p h c", c=65)[:, :, 0:64],
                            psv[:].rearrange("p (h c) -> p h c", c=HD),
                        )

            # tail inputs: c_proj slice, mlp1 weights, x q-slice. The pool
            # opens after stage 1 so it reuses the wkv space -- the WAR
            # dependency naturally defers these loads past the stage-1
            # x streams (the scheduler would otherwise hoist them).
            tailw_cm = tc.tile_pool(name=f"tailw{rep}", bufs=1)
            tailw = tailw_cm.__enter__()
            cw_sb = tailw.tile([P, 2 * H], F32R, tag="cw")
            m1_sb = tailw.tile([P, nf * H], F32R, tag="m1")
            m1b_sb = tailw.tile([1, H], F32R, tag="m1b")
            xk_sb = [tailw.tile([P, (nf // 2) * QB], F32R, tag=f"xk{i}",
                                name=f"xk{i}")
                     for i in range(2)]
            xkaug_sb = tailw.tile([1, QB], F32R, tag="xkaug")
            nc.gpsimd.dma_start(
                out=cw_sb[:].rearrange("p (t d) -> p t d", d=H),
                in_=cpw_d[:].rearrange("(t p) d -> p t d", p=P),
            )
            nc.gpsimd.dma_start(
                out=m1_sb[:].rearrange("p (t d) -> p t d", d=H),
                in_=mw1_d[:H].rearrange("(t p) d -> p t d", p=P),
            )
            nc.gpsimd.dma_start(out=m1b_sb[:], in_=mw1_d[H : H + 1])
            for i in range(2):
                nc.gpsimd.dma_start(
                    out=xk_sb[i][:].rearrange("p (t q) -> p t q", q=QB),
                    in_=xsl_d[i * (H // 2) : (i + 1) * (H // 2)]
                    .rearrange("(t p) q -> p t q", p=P),
                )
            nc.gpsimd.dma_start(out=xkaug_sb[:], in_=xsl_d[H : H + 1, :])

            # ======== stage 2: Q + attention + c_proj partial fillers ======
            # ps2 stays open through the mlp so its PSUM slots can be
            # recycled as the mlp accumulators without a pool barrier.
            with tc.tile_pool(name=f"ps2{rep}", bufs=1, space="PSUM") as ps2:
                attn_tiles = {}
                cp_fill = []   # pending c_proj filler closures

                def mk_cproj(qb, at, cpout):
                    def mk(ot):
                        def go():
                            pso = ps2.tile([P, QB], F32, tag="tq", bufs=2,
                                           name=f"cp{qb}_{ot}")
                            for p in range(2):
                                nc.tensor.matmul(
                                    pso[:],
                                    lhsT=cw_sb[
                                        :, p * H + ot * P : p * H + (ot + 1) * P
                                    ],
                                    rhs=at[:, p * QB : (p + 1) * QB],
                                    start=(p == 0),
                                    stop=(p == 1),
                                )
                            ev = cpout.tile([P, QB], F32, tag="cpev",
                                            name=f"cpev{qb}_{ot}")
                            nc.vector.tensor_copy(ev[:], pso[:])
                            nc.sync.dma_start(
                                out=zin[qb * H + ot * P : qb * H + (ot + 1) * P, :],
                                in_=ev[:],
                            )
                        return go

                    for ot in range(nf):
                        cp_fill.append(mk(ot))

                def emit_mlp1(ot, mo_ap):
                    for t in range(nf):
                        nc.tensor.matmul(
                            mo_ap,
                            lhsT=m1_sb[:, t * H + ot * P : t * H + (ot + 1) * P],
                            rhs=xk_sb[t // 4][:, (t % 4) * QB : (t % 4 + 1) * QB],
                            start=(t == 0),
                            stop=False,
                        )
                    nc.tensor.matmul(
                        mo_ap,
                        lhsT=m1b_sb[0:1, ot * P : (ot + 1) * P],
                        rhs=xkaug_sb[:],
                        start=False,
                        stop=False,
                    )

                mo_aps = [None] * nf

                with (
                    tc.tile_pool(name=f"qtp{rep}", bufs=2) as qtp,
                    tc.tile_pool(name=f"attnp{rep}", bufs=2) as attnp,
                    tc.tile_pool(name=f"cpout{rep}", bufs=2) as cpout,
                    tc.tile_pool(name=f"epool{rep}", bufs=6) as epool,
                    tc.tile_pool(name=f"small{rep}", bufs=2) as small,
                ):
                    for qb in range(nq):
                        # ---- Q^T for this q-block (streamed x) ----
                        cs = slice(qb * QB, (qb + 1) * QB)
                        x_ch = [
                            xstream.tile([P, (nf // 2) * QB], F32R, tag="xch",
                                         name=f"xq{qb}_{i}")
                            for i in range(2)
                        ]
                        x_aug = augstream.tile([1, QB], F32R, tag="xaug")
                        for i in range(2):
                            nc.sync.dma_start(
                                out=x_ch[i][:].rearrange("p (t q) -> p t q", q=QB),
                                in_=xatt[i * (H // 2) : (i + 1) * (H // 2)]
                                .rearrange("(t p) q -> p t q", p=P)[:, :, cs],
                            )
                        nc.sync.dma_start(out=x_aug[:], in_=xatt[H : H + 1, cs])
                        qt = qtp.tile([P, 2 * QB], F32R, tag="qt", name=f"qt{qb}")
                        for p in range(2):
                            ps = ps2.tile([P, QB], F32, tag="tq", bufs=2,
                                          name=f"qacc{qb}_{p}")
                            for t in range(nf):
                                nc.tensor.matmul(
                                    ps[:],
                                    lhsT=w_slice(wq_sb, t, p),
                                    rhs=x_ch[t // 4][
                                        :, (t % 4) * QB : (t % 4 + 1) * QB
                                    ],
                                    start=(t == 0),
                                    stop=False,
                                )
                            nc.tensor.matmul(
                                ps[:],
                                lhsT=wqb_sb[0:1, p * P : (p + 1) * P],
                                rhs=x_aug[:],
                                start=False,
                                stop=True,
                            )
                            nc.vector.tensor_copy(
                                qt[:, p * QB : (p + 1) * QB], ps[:]
                            )
                        # two fillers here bridge the Q->scores DVE latency
                        for _ in range(2):
                            if cp_fill:
                                cp_fill.pop(0)()

                        at = attnp.tile([P, 2 * QB], F32R, tag="attnT",
                                        name=f"attnT{qb}")
                        attn_tiles[qb] = at
                        for p in range(2):
                            pvs = [
                                ps2.tile([65, QB], F32, tag="pv", bufs=2,
                                         name=f"pv{qb}_{p}_{_h}")
                                for _h in range(2)
                            ]
                            etiles = [None, None]
                            for c2 in range(T // 2 + 1):
                                if c2 < T // 2:
                                    sstiles = [
                                        ps2.tile([P, 2 * QB], F32, tag="sc",
                                                 bufs=2, name=f"sc{_h}")
                                        for _h in range(2)
                                    ]
                                    for j in range(2):
                                        kt = 2 * c2 + j
                                        for half in range(2):
                                            nc.tensor.matmul(
                                                sstiles[half][:, j * QB : (j + 1) * QB],
                                                lhsT=KT_sb[
                                                    64 * half : 64 * half + 64,
                                                    p * SKP + kt * P : p * SKP + (kt + 1) * P,
                                                ],
                                                rhs=qt[64 * half : 64 * half + 64,
                                                       p * QB : (p + 1) * QB],
                                                start=True,
                                                stop=True,
                                                tile_position=(64 * half, 0),
                                            )
                                    enew = []
                                    for half in range(2):
                                        e = epool.tile(
                                            [P, 2 * QB], F32R, tag="e",
                                            name=f"e{half}"
                                        )
                                        nc.scalar.activation(
                                            e[:], sstiles[half][:], AF.Exp
                                        )
                                        enew.append(e)
                                # PV for the PREVIOUS chunk (software pipeline)
                                if c2 > 0:
                                    for j in range(2):
                                        kt = 2 * (c2 - 1) + j
                                        for half in range(2):
                                            h = 2 * p + half
                                            nc.tensor.matmul(
                                                pvs[half][:],
                                                lhsT=V_sb[
                                                    :,
                                                    kt * 260 + h * 65 : kt * 260
                                                    + (h + 1) * 65,
                                                ],
                                                rhs=etiles[half][:, j * QB : (j + 1) * QB],
                                                start=(kt == 0),
                                                stop=False,
                                            )
                                if c2 < T // 2:
                                    etiles = enew
                                # c_proj filler between chunks
                                if cp_fill and c2 >= 1:
                                    cp_fill.pop(0)()
                            # masked-key rank-1 correction closes each PV
                            for half in range(2):
                                h = 2 * p + half
                                nc.tensor.matmul(
                                    pvs[half][:],
                                    lhsT=corr_sb[0:1, h * 65 : (h + 1) * 65],
                                    rhs=ones_sb[:],
                                    start=False,
                                    stop=True,
                                )
                            # normalize by the denominator (row 64), store attn^T
                            for half in range(2):
                                rec = small.tile([1, QB], F32, tag="rec")
                                nc.vector.reciprocal(rec[:], pvs[half][64:65, :])
                                recb = small.tile([64, QB], F32, tag="recb")
                                nc.gpsimd.partition_broadcast(
                                    recb[:], rec[:], channels=64
                                )
                                nc.vector.tensor_tensor(
                                    at[64 * half : 64 * half + 64,
                                       p * QB : (p + 1) * QB],
                                    pvs[half][0:64, :],
                                    recb[:],
                                    ALU.mult,
                                )
                        mk_cproj(qb, attn_tiles.pop(qb), cpout)

                    # ---- tail of stage 2: interleave the last q-block's
                    # c_proj with mlp1; accumulators recycle ps2 slots ----
                    moA = ps2.tile([P, 2 * QB], F32, tag="sc", bufs=2,
                                   name="moA")
                    moB = ps2.tile([P, 2 * QB], F32, tag="sc", bufs=2,
                                   name="moB")
                    mo4 = ps2.tile([P, QB], F32, tag="pv", bufs=2, name="mo4")
                    mo5 = ps2.tile([P, QB], F32, tag="pv", bufs=2, name="mo5")
                    mo_aps[0] = moA[:, 0:QB]
                    mo_aps[1] = moA[:, QB : 2 * QB]
                    mo_aps[2] = moB[:, 0:QB]
                    mo_aps[3] = moB[:, QB : 2 * QB]
                    mo_aps[4] = mo4[:]
                    mo_aps[5] = mo5[:]
                    for ot in range(nf):
                        if cp_fill:
                            cp_fill.pop(0)()
                        if ot == 6:
                            mo6 = ps2.tile([P, QB], F32, tag="tq", bufs=2,
                                           name="mo6")
                            mo_aps[6] = mo6[:]
                        if ot == 7:
                            mo7 = ps2.tile([P, QB], F32, tag="tq", bufs=2,
                                           name="mo7")
                            mo_aps[7] = mo7[:]
                        emit_mlp1(ot, mo_aps[ot])
                    assert not cp_fill

                # ---------- ReduceScatter: z partials -> own q-slice ------
                if cc_mode == "cc":
                    nc.gpsimd.collective_compute(
                        "ReduceScatter", ALU.add, replica_groups=rg,
                        ins=[zin[:].opt()], outs=[zred[:].opt()],
                    )
                else:
                    # timing experiment: same output bytes, no collective
                    nc.sync.dma_start(out=zred[:], in_=zin[0:H, :])

                # ---------- mlp2 (after the collective) ------
                with (
                    tc.tile_pool(name=f"m2p{rep}", bufs=1) as m2p,
                    tc.tile_pool(name=f"mt{rep}", bufs=1) as mt,
                    tc.tile_pool(name=f"gout{rep}", bufs=4) as goutp,
                ):
                    m2_sb = m2p.tile([P, nf * H], F32R, tag="m2")
                    nc.gpsimd.dma_start(
                        out=m2_sb[:].rearrange("p (t d) -> p t d", d=H),
                        in_=mw2_d[:].rearrange("(t p) d -> p t d", p=P),
                    )
                    # z^T slice back from the collective
                    zch = mt.tile([P, nf * QB], F32R, tag="zch")
                    for ct in range(nf):
                        nc.sync.dma_start(
                            out=zch[:, ct * QB : (ct + 1) * QB],
                            in_=zred[ct * P : (ct + 1) * P, :].bitcast(F32R),
                        )
                    # mlp2: z-half, accumulate into the same PSUM banks.
                    # ct-major so the first matmuls start as soon as the
                    # first zch chunk lands; the last ct per ot is fused
                    # with its gelu+store so evacuation pipelines.
                    for ct in range(nf - 1):
                        for ot in range(nf):
                            nc.tensor.matmul(
                                mo_aps[ot],
                                lhsT=m2_sb[:, ct * H + ot * P : ct * H + (ot + 1) * P],
                                rhs=zch[:, ct * QB : (ct + 1) * QB],
                                start=False,
                                stop=False,
                            )
                    ct = nf - 1
                    for ot in range(nf):
                        nc.tensor.matmul(
                            mo_aps[ot],
                            lhsT=m2_sb[:, ct * H + ot * P : ct * H + (ot + 1) * P],
                            rhs=zch[:, ct * QB : (ct + 1) * QB],
                            start=False,
                            stop=True,
                        )
                        gout = goutp.tile([P, QB], F32, tag="gout",
                                          name=f"gout{ot}")
                        nc.scalar.activation(gout[:], mo_aps[ot], AF.Gelu_apprx_tanh)
                        nc.sync.dma_start(
                            out=outT[ot * P : (ot + 1) * P, :],
                            in_=gout[:],
                        )
            tailw_cm.__exit__(None, None, None)

    nc.compile()
    return nc


def w_slice(w_sb, t, p):
    """lhsT [128, 128] slice: f-tile t, output half p, of a [128, nt*256] layout."""
    return w_sb[:, t * DH + p * P : t * DH + (p + 1) * P]


_NC_CACHE = {}
LAST_RESULTS = None


def _get_nc(T):
    key = ("T", T)
    if key not in _NC_CACHE:
        _NC_CACHE[key] = _build_nc(T)
    return _NC_CACHE[key]


def _get_nc_reps(reps, T=None):
    if T is None:
        # reference setup_inputs mask (seed 0) -> use the common case
        T = _pick_T_default()
    key = ("reps", reps, T)
    if key not in _NC_CACHE:
        _NC_CACHE[key] = _build_nc(T, reps=reps)
    return _NC_CACHE[key]


def _pick_T(mask):
    """Even number of 128-wide key tiles covering the unmasked keys of
    every batch (same NEFF on all cores)."""
    n_max = int(max((mask[b] != 0).sum() for b in range(mask.shape[0])))
    return max(2, 2 * ((n_max + 255) // 256))


def _pick_T_default():
    import reference  # only available in the dev environment
    m = np.asarray(reference.setup_inputs()["attendee_mask"])
    return _pick_T(m)


def kernel(**inputs):
    global LAST_RESULTS
    mask = np.asarray(inputs["attendee_mask"])
    T = _pick_T(mask)
    nc = _get_nc(T)
    in_maps = make_in_maps(inputs, T)

    trace = bool(int(os.environ.get("KERNEL_TRACE", "0")))
    res = bass_utils.run_bass_kernel_spmd(
        nc, in_maps, core_ids=list(range(NCORES)), trace=trace
    )
    LAST_RESULTS = res

    out = np.empty((B, S, H), np.float32)
    for c in range(NCORES):
        b, g = c // TP, c % TP
        out[b, g * SQ4 : (g + 1) * SQ4, :] = res.results[c]["outT"].T
    return out


def make_in_maps(inputs, T):
    SKP = T * P
    xq = np.ascontiguousarray(np.asarray(inputs["attender_seq"], np.float32))
    xk = np.ascontiguousarray(np.asarray(inputs["attendee_seq"], np.float32))
    mask = np.asarray(inputs["attendee_mask"])
    caw = np.asarray(inputs["c_attn_w"], np.float32)
    cab = np.asarray(inputs["c_attn_b"], np.float32)
    cpw = np.ascontiguousarray(np.asarray(inputs["c_proj_w"], np.float32))
    cpb = np.asarray(inputs["c_proj_b"], np.float32)
    mw = np.ascontiguousarray(np.asarray(inputs["mlp_w"], np.float32))
    mb = np.asarray(inputs["mlp_b"], np.float32)

    # fold the c_proj bias into the mlp bias: cat(x, z0+cpb)@W + mb
    #   = x@W1 + z0@W2 + (mb + cpb@W2)
    mb_eff = mb + cpb @ mw[H:, :]
    mw1 = np.concatenate([mw[:H, :], mb_eff[None, :]], 0)   # [H+1, H]
    mw2 = mw[H:, :]                                         # [H, H]

    # compacted (unmasked-first, zero-padded) attendee data per batch
    xatd_perm = []
    xsum_aug = []
    n_masked = []
    n_pad = []
    for b in range(B):
        m = mask[b] != 0
        idx = np.nonzero(m)[0]
        n_un = len(idx)
        xt = np.concatenate([xk[b].T, np.ones((1, S), np.float32)], 0)
        xp = np.zeros((H + 1, SKP), np.float32)
        xp[:, :n_un] = xt[:, idx]
        xatd_perm.append(xp)
        xsum_aug.append(xt[:, ~m].sum(axis=1))   # [H+1], aug row -> count
        n_masked.append(S - n_un)
        n_pad.append(SKP - n_un)

    in_maps = []
    for c in range(NCORES):
        b, g = c // TP, c % TP
        gs = slice(g * DH, (g + 1) * DH)
        xattT = np.concatenate([xq[b].T, np.ones((1, S), np.float32)], 0)
        wq = np.concatenate([caw[:, gs], cab[None, gs]], 0)
        wk = np.concatenate(
            [caw[:, H + g * DH : H + (g + 1) * DH],
             cab[None, H + g * DH : H + (g + 1) * DH]], 0)
        wv = np.concatenate(
            [caw[:, 2 * H + g * DH : 2 * H + (g + 1) * DH],
             cab[None, 2 * H + g * DH : 2 * H + (g + 1) * DH]], 0)
        # masked-key correction row: per head [sum_masked V (64),
        # count_masked - n_padding]
        vsum = xsum_aug[b] @ wv                  # [DH]; aug entry * bias ok
        corr = np.empty((1, 4 * 65), np.float32)
        for h in range(HPC):
            corr[0, h * 65 : h * 65 + 64] = vsum[h * 64 : (h + 1) * 64]
            corr[0, h * 65 + 64] = n_masked[b] - n_pad[b]
        in_maps.append({
            "xatt": np.ascontiguousarray(xattT),
            "xatd": np.ascontiguousarray(xatd_perm[b]),
            "xslice": np.ascontiguousarray(
                xattT[:, g * SQ4 : (g + 1) * SQ4]),
            "wq": np.ascontiguousarray(wq),
            "wk": np.ascontiguousarray(wk),
            "wv": np.ascontiguousarray(wv),
            "corr": corr,
            "cprojw": np.ascontiguousarray(cpw[gs, :]),
            "mlpw1": np.ascontiguousarray(mw1),
            "mlpw2": np.ascontiguousarray(mw2),
        })
    return in_maps
